# revision 25
# baseline (speedup 1.0000x reference)
"""Bass/Trainium2 kernel for a 6-layer GPT-style transformer (BigramLanguageModel).

Contract: kernel(**inputs) takes the FULL unsharded inputs from
reference.setup_inputs() and returns the FULL [32, 512, 65] fp32 logits.

Sharding: data-parallel over batch. Each of the 8 NeuronCores runs the whole
model on 4 of the 32 sequences (params replicated); outputs are concatenated
on the host. No collectives.

Device-side design (per core, 2048 tokens), v2 -- tuned for continuous PE
streaming (the v1 kernel spent ~500us at HAM K=4/8 half-clock because the
attention inner loop left the PE micro-idle):
 - residual x: token-major SBUF [128, 16, 384] fp32. LayerNorm stats are
   produced *inside* the preceding residual-add loop (tensor_tensor_reduce
   gives sum(x) for free, an ACT Square/DVE pass gives sum(x^2)), rstd is
   computed as exp(-0.5*ln(var+eps)) so ACT only ever needs the
   exp_and_others table (no ~2.7us table reloads), applies run on GPSIMD,
   and the PE transposes (fp32r: 1.5 cyc/row vs 2.0 for fp32) interleave
   with the previous phase's matmuls.
 - hT/h2T/xfT, otc, mlpT, v_aug, q/k are bf16 (activations only; weights
   stay fp32r): halves LDWEIGHTS time for activation-stationary matmuls and
   keeps every matmul at 1.0 cycles/row (fp32r pays 4x below 256 free).
 - attention: 2 same-parity (head, seq) units per group share one
   [128,1024] PSUM tile pair; exp is batched across the 2 units straight
   out of PSUM; the causal mask is applied by an extra 128-col PE matmul
   (-1e30 * strict-lower-tri) accumulated into S before exp, so the
   S->exp->AV chain crosses only one non-PE engine. V-augmentation is
   uniform [ones64 | V64] per head so the softmax denominator always lands
   in PSUM rows 0:64 (reciprocal_approx_fast needs partition-0 input).
 - independent work (V matmuls, next pair's QT/KT, early proj tiles) is
   emitted between attention groups as PE filler to keep HAM at K=8/8.
 - MLP: W1 -> relu copies alternating ACT/DVE -> bf16 mlpT -> W2; weight
   DMAs for the whole layer issue at layer top (prefetch under attention).
"""

import sys

for _p in ("/opt/trn_rl_repo", "/opt/pypackages"):
    if _p not in sys.path:
        sys.path.insert(0, _p)

import numpy as np
import ml_dtypes

import concourse.bass as bass
import concourse.tile as tile
from concourse import bacc, mybir
from concourse.bass_utils import run_bass_kernel_spmd

F32 = mybir.dt.float32
F32R = mybir.dt.float32r
BF16 = mybir.dt.bfloat16

N_EMBED = 384
CONTEXT = 512
N_HEADS = 6
HEAD_DIM = 64
N_LAYERS = 6
VOCAB = 65
B, T = 32, 512
LN_EPS = 1e-5
N_CORES = 8
B_LOC = B // N_CORES          # 4 sequences per core
N_TOK = B_LOC * T             # 2048 tokens per core
N_TILES = N_TOK // 128        # 16 token tiles
N_CHUNKS = N_EMBED // 128     # 3 E-chunks
N_MLP = 4 * N_EMBED           # 1536
N_MCHUNK = N_MLP // 128       # 12
SCALE = float(N_EMBED) ** -0.5
MDT = F32R
NEG_BIG = -1.0e30
V_W = N_HEADS * 128           # [ones64 | V64] per head -> 768 cols


def _prep(inputs):
    """Host-side layout prep + exact LN folds. Returns (shared, has, per_core_idx)."""
    f = lambda a: np.ascontiguousarray(np.asarray(a), dtype=np.float32)
    idx = np.asarray(inputs["idx"])
    tok_emb, pos_emb = f(inputs["tok_emb"]), f(inputs["pos_emb"])
    Wq, Wk, Wv = f(inputs["Wq"]), f(inputs["Wk"]), f(inputs["Wv"])
    Wproj, bproj = f(inputs["Wproj"]), f(inputs["bproj"])
    W1, b1, W2, b2 = f(inputs["W1"]), f(inputs["b1"]), f(inputs["W2"]), f(inputs["b2"])
    ln1_g, ln1_b = f(inputs["ln1_g"]), f(inputs["ln1_b"])
    ln2_g, ln2_b = f(inputs["ln2_g"]), f(inputs["ln2_b"])
    lnf_g, lnf_b = f(inputs["lnf_g"]), f(inputs["lnf_b"])
    Wlm, blm = f(inputs["Wlm"]), f(inputs["blm"])

    L, H, E, D = N_LAYERS, N_HEADS, N_EMBED, HEAD_DIM

    # fold ln gains into the consuming weights (exact)
    Wq_f = ln1_g[:, None, :, None] * Wq          # [L,H,E,D]
    Wk_f = ln1_g[:, None, :, None] * Wk
    Wv_f = ln1_g[:, None, :, None] * Wv
    W1_f = ln2_g[:, :, None] * W1                # [L,E,4E]
    Wlm_f = lnf_g[:, None] * Wlm                 # [E,V]

    # ln biases propagate through the matmuls as constant bias vectors
    qb = np.einsum("le,lhed->lhd", ln1_b, Wq)    # [L,H,D]
    kb = np.einsum("le,lhed->lhd", ln1_b, Wk)
    vb = np.einsum("le,lhed->lhd", ln1_b, Wv)
    b1_eff = b1 + np.einsum("le,lem->lm", ln2_b, W1)    # [L,4E]
    blm_eff = blm + lnf_b @ Wlm                          # [V]

    # head-pair packed QT/KT weights: [L, 3, E, 128]  (pair r = heads 2r, 2r+1)
    wqp = np.concatenate([Wq_f[:, 0::2], Wq_f[:, 1::2]], axis=-1)  # [L,3,E,128]
    wkp = np.concatenate([Wk_f[:, 0::2], Wk_f[:, 1::2]], axis=-1)
    qbp = np.concatenate([qb[:, 0::2], qb[:, 1::2]], axis=-1)      # [L,3,128]
    kbp = np.concatenate([kb[:, 0::2], kb[:, 1::2]], axis=-1)
    wv_all = Wv_f.transpose(0, 2, 1, 3).reshape(L, E, H * D)       # [L,E,384]
    vb_all = vb.reshape(L, H * D)

    # causal mask as additive matmul rhs: -BIG on strict lower triangle (k > j)
    trib = (np.tril(np.ones((128, 128), dtype=np.float32), -1) * NEG_BIG)

    shared = dict(
        tok_emb=tok_emb,
        pos_emb=pos_emb,
        wqp=np.ascontiguousarray(wqp.astype(ml_dtypes.bfloat16)),
        wkp=np.ascontiguousarray(wkp.astype(ml_dtypes.bfloat16)),
        wv=np.ascontiguousarray(wv_all.astype(ml_dtypes.bfloat16)),
        wp=np.ascontiguousarray(Wproj.astype(ml_dtypes.bfloat16)),
        w1=np.ascontiguousarray(W1_f.astype(ml_dtypes.bfloat16)),
        w2=np.ascontiguousarray(W2.astype(ml_dtypes.bfloat16)),
        wlm=np.ascontiguousarray(Wlm_f.astype(ml_dtypes.bfloat16)),
        ident=np.eye(128, dtype=np.float32),
        iota=np.arange(VOCAB, dtype=np.float32).reshape(VOCAB, 1),
        trib=np.ascontiguousarray(trib.astype(ml_dtypes.bfloat16)),
    )
    flags = dict(
        qb=qbp if np.any(qbp) else None,
        kb=kbp if np.any(kbp) else None,
        vb=np.broadcast_to(vb_all[:, None, :], (L, 128, H * D)).copy()
        if np.any(vb) else None,
        bp=np.broadcast_to(bproj[:, None, :], (L, 128, E)).copy()
        if np.any(bproj) else None,
        b1=np.ascontiguousarray(b1_eff.reshape(L, N_MCHUNK, 128).transpose(0, 2, 1))
        if np.any(b1_eff) else None,                    # [L,128,12] partition-major
        b2=np.broadcast_to(b2[:, None, :], (L, 128, E)).copy() if np.any(b2) else None,
        blm=np.broadcast_to(blm_eff[None, :], (128, VOCAB)).copy()
        if np.any(blm_eff) else None,
    )
    for k, v in flags.items():
        if v is not None:
            shared[k] = np.ascontiguousarray(v, dtype=np.float32)
    has = {k: (v is not None) for k, v in flags.items()}

    idx_f = idx.astype(np.float32).reshape(N_CORES, N_TOK)
    return shared, has, idx_f


def _build(has):
    nc = bacc.Bacc(trn_type="TRN2", debug=False, num_devices=N_CORES)
    d = {}
    d["idxf"] = nc.dram_tensor("idxf", [N_TOK], F32, kind="ExternalInput")
    d["tok_emb"] = nc.dram_tensor("tok_emb", [VOCAB, N_EMBED], MDT, kind="ExternalInput")
    d["pos_emb"] = nc.dram_tensor("pos_emb", [CONTEXT, N_EMBED], F32, kind="ExternalInput")
    d["wqp"] = nc.dram_tensor("wqp", [N_LAYERS, 3, N_EMBED, 128], BF16, kind="ExternalInput")
    d["wkp"] = nc.dram_tensor("wkp", [N_LAYERS, 3, N_EMBED, 128], BF16, kind="ExternalInput")
    d["wv"] = nc.dram_tensor("wv", [N_LAYERS, N_EMBED, N_EMBED], BF16, kind="ExternalInput")
    d["wp"] = nc.dram_tensor("wp", [N_LAYERS, N_EMBED, N_EMBED], BF16, kind="ExternalInput")
    d["w1"] = nc.dram_tensor("w1", [N_LAYERS, N_EMBED, N_MLP], BF16, kind="ExternalInput")
    d["w2"] = nc.dram_tensor("w2", [N_LAYERS, N_MLP, N_EMBED], BF16, kind="ExternalInput")
    d["wlm"] = nc.dram_tensor("wlm", [N_EMBED, VOCAB], BF16, kind="ExternalInput")
    d["ident"] = nc.dram_tensor("ident", [128, 128], F32, kind="ExternalInput")
    d["iota"] = nc.dram_tensor("iota", [VOCAB, 1], F32, kind="ExternalInput")
    d["trib"] = nc.dram_tensor("trib", [128, 128], BF16, kind="ExternalInput")
    if has["qb"]:
        d["qb"] = nc.dram_tensor("qb", [N_LAYERS, 3, 128], F32, kind="ExternalInput")
    if has["kb"]:
        d["kb"] = nc.dram_tensor("kb", [N_LAYERS, 3, 128], F32, kind="ExternalInput")
    if has["vb"]:
        d["vb"] = nc.dram_tensor("vb", [N_LAYERS, 128, N_EMBED], F32, kind="ExternalInput")
    if has["bp"]:
        d["bp"] = nc.dram_tensor("bp", [N_LAYERS, 128, N_EMBED], F32, kind="ExternalInput")
    if has["b1"]:
        d["b1"] = nc.dram_tensor("b1", [N_LAYERS, 128, N_MCHUNK], F32, kind="ExternalInput")
    if has["b2"]:
        d["b2"] = nc.dram_tensor("b2", [N_LAYERS, 128, N_EMBED], F32, kind="ExternalInput")
    if has["blm"]:
        d["blm"] = nc.dram_tensor("blm", [128, VOCAB], F32, kind="ExternalInput")
    logits_d = nc.dram_tensor("logits", [N_TOK, VOCAB], F32, kind="ExternalOutput")

    AF = mybir.ActivationFunctionType
    OP = mybir.AluOpType

    with tile.TileContext(nc) as tc:
        with tc.tile_pool(name="const", bufs=1) as cst, \
             tc.tile_pool(name="persist", bufs=1) as per, \
             tc.tile_pool(name="work", bufs=3) as wk, \
             tc.tile_pool(name="htile", bufs=4) as hp, \
             tc.tile_pool(name="wts", bufs=4) as wts, \
             tc.tile_pool(name="psA", bufs=2, space="PSUM") as psA, \
             tc.tile_pool(name="psB", bufs=2, space="PSUM") as psB:

            # ---- constants ----
            ident = cst.tile([128, 128], F32)
            nc.sync.dma_start(ident, d["ident"][:, :])
            iota = cst.tile([VOCAB, 1], F32)
            nc.sync.dma_start(iota, d["iota"][:, :])
            trib = cst.tile([128, 128], BF16)
            nc.sync.dma_start(trib, d["trib"][:, :])
            eps_sb = cst.tile([128, 1], F32)
            nc.vector.memset(eps_sb, LN_EPS)
            tok_sb = cst.tile([VOCAB, N_EMBED], MDT)
            nc.sync.dma_start(tok_sb, d["tok_emb"][:, :])
            identb = cst.tile([128, 128], BF16)
            nc.vector.tensor_copy(identb, ident)
            tri01 = cst.tile([128, 128], BF16)
            nc.vector.tensor_scalar(out=tri01, in0=trib, scalar1=0.0,
                                    scalar2=None, op0=OP.is_equal)

            bias_sb = {}
            for nm in ("vb", "bp", "b2"):
                if has[nm]:
                    bias_sb[nm] = cst.tile([128, N_LAYERS, N_EMBED], F32)
                    nc.sync.dma_start(bias_sb[nm], d[nm].rearrange("l p e -> p l e"))
            if has["b1"]:
                bias_sb["b1"] = cst.tile([128, N_LAYERS, N_MCHUNK], F32)
                nc.sync.dma_start(bias_sb["b1"], d["b1"].rearrange("l p m -> p l m"))
            for nm in ("qb", "kb"):
                if has[nm]:
                    bias_sb[nm] = cst.tile([128, N_LAYERS, 3], F32)
                    nc.sync.dma_start(bias_sb[nm], d[nm].rearrange("l r p -> p l r"))
            if has["blm"]:
                bias_sb["blm"] = cst.tile([128, VOCAB], F32)
                nc.sync.dma_start(bias_sb["blm"], d["blm"][:, :])

            # ---- persistent activations ----
            x = per.tile([128, N_TILES, N_EMBED], F32)          # residual, token-major
            v_aug = per.tile([128, N_TILES, V_W], BF16)         # [ones64|V64] per head
            ones_blk = cst.tile([128, 64], F32)
            nc.vector.memset(ones_blk, 1.0)
            nc.vector.tensor_copy(
                v_aug.rearrange("p t (h j) -> p t h j", h=N_HEADS)[:, :, :, 0:64],
                ones_blk[:, None, None, :].to_broadcast(
                    [128, N_TILES, N_HEADS, 64]))

            # round-robin engine pickers for PSUM->SBUF copies
            _rr = {"c": 0}

            def copy_out(dst, src):
                _rr["c"] += 1
                if _rr["c"] % 2 == 0:
                    nc.scalar.copy(dst, src)
                else:
                    nc.vector.tensor_copy(dst, src)

            # =========================================================
            # LayerNorm pass (baseline-proven shape; rstd via Ln/Exp so
            # ACT stays on the exp_and_others table all kernel long)
            # =========================================================
            def ln_stats_group(nm, tg):
                    mv4 = wk.tile([128, 4, 2], F32, tag="mv" + nm)
                    for dt_ in range(4):
                        st = wk.tile([128, 6], F32, tag="bnst")
                        nc.vector.bn_stats(out=st, in_=x[:, tg * 4 + dt_, :])
                        nc.vector.bn_aggr(out=mv4[:, dt_, :], in_=st)
                    sstd = wk.tile([128, 4], F32, tag="sstd")
                    nc.scalar.activation(out=sstd, in_=mv4[:, :, 1],
                                         func=AF.Sqrt, bias=eps_sb, scale=1.0)
                    rstd = wk.tile([128, 4], F32, tag="rstd")
                    nc.vector.reciprocal(out=rstd, in_=sstd)
                    return mv4, rstd

            def ln_apply_group(state, tg):
                    mv4, rstd = state
                    hts = []
                    for dt_ in range(4):
                        t = tg * 4 + dt_
                        ht = hp.tile([128, N_EMBED], BF16, tag="h", bufs=8)
                        nc.vector.tensor_scalar(
                            out=ht, in0=x[:, t, :],
                            scalar1=mv4[:, dt_, 0:1],
                            scalar2=rstd[:, dt_:dt_ + 1],
                            op0=OP.subtract, op1=OP.mult)
                        hts.append(ht)
                    return hts

            def ln_tp_group(dst_hT, hts, tg):
                    for c in range(N_CHUNKS):
                        pt = psA.tile([128, 512], F32, tag="gen")
                        ptb = pt.bitcast(BF16)
                        for dt_ in range(4):
                            nc.tensor.transpose(
                                ptb[:, dt_ * 128:(dt_ + 1) * 128],
                                hts[dt_][:, c * 128:(c + 1) * 128], identb)
                        nc.scalar.copy(dst_hT[:, c, tg * 512:(tg + 1) * 512],
                                       ptb[:, 0:512])

            def ln_group(dst_hT, nm, tg):
                    stt = ln_stats_group(nm, tg)
                    hts = ln_apply_group(stt, tg)
                    ln_tp_group(dst_hT, hts, tg)

            def layernorm_to(dst_hT, nm):
                for tg in range(N_TILES // 4):
                    ln_group(dst_hT, nm, tg)

            def make_ln_pipe(dst_hT, nm):
                state = {"st": {}, "hts": {}}

                def on_group(tg):
                    state["st"][tg] = ln_stats_group(nm, tg)
                    if tg >= 1:
                        state["hts"][tg - 1] = ln_apply_group(
                            state["st"].pop(tg - 1), tg - 1)
                    if tg >= 2:
                        ln_tp_group(dst_hT, state["hts"].pop(tg - 2), tg - 2)

                def flush():
                    state["hts"][3] = ln_apply_group(state["st"].pop(3), 3)
                    ln_tp_group(dst_hT, state["hts"].pop(2), 2)
                    ln_tp_group(dst_hT, state["hts"].pop(3), 3)

                return on_group, flush

            # =========================================================
            # embedding: x = onehot(idx) @ tok_emb + pos; LN1 of layer 0
            # =========================================================
            for t in range(N_TILES):
                idx_b = wk.tile([VOCAB, 128], F32, tag="idxb")
                nc.sync.dma_start(
                    idx_b,
                    bass.AP(tensor=d["idxf"], offset=t * 128,
                            ap=[[0, VOCAB], [1, 128]]))
                oh = wk.tile([VOCAB, 128], MDT, tag="oh")
                nc.vector.tensor_scalar(out=oh, in0=idx_b, scalar1=iota,
                                        scalar2=None, op0=OP.is_equal)
                pe = psA.tile([128, 512], F32, tag="gen")
                nc.tensor.matmul(pe[:, :N_EMBED], lhsT=oh,
                                 rhs=tok_sb, start=True, stop=True)
                nc.scalar.copy(x[:, t, :], pe[:, :N_EMBED])
                nc.gpsimd.dma_start(
                    out=x[:, t, :],
                    in_=d["pos_emb"][(t % 4) * 128:(t % 4) * 128 + 128, :],
                    accum_op=OP.add)

            for layer in range(N_LAYERS):
                hT = per.tile([128, N_CHUNKS, N_TOK], BF16, tag="ht", bufs=2)
                layernorm_to(hT, "1")
                # ---- prefetch all weights of this layer ----
                wv_c, wp_c = [], []
                for c in range(N_CHUNKS):
                    w = wts.tile([128, N_EMBED], BF16, tag="wvchk", bufs=3)
                    nc.sync.dma_start(w, d["wv"][layer, c * 128:(c + 1) * 128, :])
                    wv_c.append(w)
                for c in range(N_CHUNKS):
                    w = wts.tile([128, N_EMBED], BF16, tag="wpchk", bufs=3)
                    nc.sync.dma_start(w, d["wp"][layer, c * 128:(c + 1) * 128, :])
                    wp_c.append(w)
                w1all = wts.tile([128, N_CHUNKS, N_MLP], BF16, tag="w1all", bufs=1)
                for c in range(N_CHUNKS):
                    nc.sync.dma_start(
                        w1all[:, c, :], d["w1"][layer, c * 128:(c + 1) * 128, :])
                w2all = wts.tile([128, N_MCHUNK, N_EMBED], BF16, tag="w2all", bufs=1)
                for m in range(N_MCHUNK):
                    nc.sync.dma_start(
                        w2all[:, m, :], d["w2"][layer, m * 128:(m + 1) * 128, :])

                otc = per.tile([128, N_CHUNKS, N_TOK], BF16, tag="big")

                # ---- QT/KT chunk emitters ----
                def emit_qk_chunks(pair):
                    qkt, chunks = {}, []
                    for nm, wd, bias_nm in (("q", d["wqp"], "qb"),
                                            ("k", d["wkp"], "kb")):
                        wqk = wts.tile([128, N_CHUNKS, 128], BF16, tag="wqk",
                                       bufs=2, name=f"wqk_{nm}")
                        for c in range(N_CHUNKS):
                            nc.sync.dma_start(
                                wqk[:, c, :],
                                wd[layer, pair, c * 128:(c + 1) * 128, :])
                        # rows 0:64 = even head (sub0), 64:128 = odd head
                        dstT = per.tile([128, N_TOK], BF16, tag="qk" + nm,
                                        bufs=2, name=f"qk_{nm}")
                        qkt[nm] = dstT

                        def chunk(n, wqk=wqk, dstT=dstT, bias_nm=bias_nm):
                            pq = psA.tile([128, 512], F32, tag="gen", name="pq")
                            for c in range(N_CHUNKS):
                                nc.tensor.matmul(
                                    pq, lhsT=wqk[:, c, :],
                                    rhs=hT[:, c, n * 512:(n + 1) * 512],
                                    start=(c == 0), stop=(c == N_CHUNKS - 1))
                            dst = dstT[:, n * 512:(n + 1) * 512]
                            if has[bias_nm]:
                                nc.scalar.activation(
                                    out=dst, in_=pq, func=AF.Identity,
                                    bias=bias_sb[bias_nm][:, layer, pair:pair + 1],
                                    scale=1.0)
                            else:
                                copy_out(dst, pq)

                        for n in range(N_TOK // 512):
                            chunks.append(lambda n=n, chunk=chunk: chunk(n))
                    return qkt, chunks

                # ---- V tile emitter ----
                def emit_v(t):
                    pv = psA.tile([128, 512], F32, tag="gen")
                    for c in range(N_CHUNKS):
                        nc.tensor.matmul(pv[:, :N_EMBED],
                                         lhsT=hT[:, c, t * 128:(t + 1) * 128],
                                         rhs=wv_c[c],
                                         start=(c == 0), stop=(c == N_CHUNKS - 1))
                    src = pv[:, :N_EMBED].rearrange("p (h j) -> p h j", h=N_HEADS)
                    dst = v_aug[:, t, :].rearrange(
                        "p (h j) -> p h j", h=N_HEADS)[:, :, 64:128]
                    if has["vb"]:
                        nc.vector.tensor_tensor(
                            out=dst, in0=src,
                            in1=bias_sb["vb"][:, layer, :].rearrange(
                                "p (h j) -> p h j", h=N_HEADS),
                            op=OP.add)
                    else:
                        copy_out(dst, src)

                # ---- proj tile emitter ----
                def emit_proj(t):
                    pp = psA.tile([128, 512], F32, tag="gen")
                    for c in range(N_CHUNKS):
                        nc.tensor.matmul(pp[:, :N_EMBED],
                                         lhsT=otc[:, c, t * 128:(t + 1) * 128],
                                         rhs=wp_c[c],
                                         start=(c == 0), stop=(c == N_CHUNKS - 1))
                    if has["bp"]:
                        tmp = hp.tile([128, N_EMBED], F32, tag="h")
                        nc.vector.tensor_tensor(out=tmp, in0=pp[:, :N_EMBED],
                                                in1=bias_sb["bp"][:, layer, :],
                                                op=OP.add)
                        nc.vector.tensor_tensor(out=x[:, t, :], in0=tmp,
                                                in1=x[:, t, :], op=OP.add)
                    else:
                        nc.vector.tensor_tensor(out=x[:, t, :],
                                                in0=pp[:, :N_EMBED],
                                                in1=x[:, t, :], op=OP.add)

                # ---- attention ----
                qkt, chunks0 = emit_qk_chunks(0)
                for ch in chunks0:
                    ch()
                for t in range(8):          # V for seqs 0,1 upfront
                    emit_v(t)

                fillers = []

                def pop_filler():
                    if fillers:
                        fillers.pop(0)()

                for pair in range(3):
                    if pair == 0:
                        fillers = [lambda t=t: emit_v(t) for t in range(8, 16)]
                        qkt_n, chunks_n = emit_qk_chunks(1)
                        fillers += chunks_n
                    elif pair == 1:
                        qkt_n, chunks_n = emit_qk_chunks(2)
                        fillers = list(chunks_n)
                    else:
                        fillers = []

                    # groups: same head-parity, adjacent seqs -> shared
                    # psum/exp/recip/normalize across the 2 units
                    groups = [(0, 0), (1, 0), (0, 2), (1, 2)]
                    for gi, (sub, s0) in enumerate(groups):
                        pos2 = psB.tile([128, 1024], F32, tag="ot", bufs=1,
                                        name=f"pos_{gi}")
                        for ki in range(4):
                            width = 512 - ki * 128
                            pa2 = psB.tile([128, 1024], F32, tag="at",
                                           name=f"pa_{gi}_{ki}")
                            at2 = wk.tile([128, 2, 512], BF16, tag="at_sb",
                                          bufs=4, name=f"at_{gi}_{ki}")
                            for j in range(2):
                                s = s0 + j
                                kc = s * 512 + ki * 128
                                nc.tensor.matmul(
                                    pa2[:, j * 512:j * 512 + width],
                                    lhsT=qkt["k"][64 * sub:64 * sub + 64,
                                                  kc:kc + 128],
                                    rhs=qkt["q"][64 * sub:64 * sub + 64,
                                                 kc:s * 512 + 512],
                                    start=True, stop=True)
                            nc.scalar.activation(
                                out=at2[:, :, :width],
                                in_=pa2.rearrange(
                                    "p (u w) -> p u w", u=2)[:, :, :width],
                                func=AF.Exp, scale=SCALE)
                            nc.vector.tensor_tensor(
                                out=at2[:, :, 0:128], in0=at2[:, :, 0:128],
                                in1=tri01[:, None, :].to_broadcast(
                                    [128, 2, 128]),
                                op=OP.mult)
                            for j in range(2):
                                s = s0 + j
                                h = 2 * pair + sub
                                nc.tensor.matmul(
                                    pos2[:, j * 512 + ki * 128:j * 512 + 512],
                                    lhsT=v_aug[:, s * 4 + ki,
                                               h * 128:(h + 1) * 128],
                                    rhs=at2[:, j, :width],
                                    start=(ki == 0), stop=(ki == 3))
                            pop_filler()
                        # denominator is always rows 0:64 ([ones|V] layout)
                        rho = wk.tile([64, 1024], F32, tag="rho", bufs=2,
                                      name=f"rho_{gi}")
                        nc.vector.reciprocal_approx_fast(
                            out=rho, in_=pos2[0:64, :])
                        nc.vector.tensor_tensor(
                            out=otc[64 * sub:64 * sub + 64, pair,
                                    s0 * 512:(s0 + 2) * 512],
                            in0=pos2[64:128, :], in1=rho, op=OP.mult)
                        if pair == 2 and gi == 1:
                            fillers += [lambda t=t: emit_proj(t)
                                        for t in range(8)]
                    # drain fillers before the next pair needs its QT/KT
                    while fillers:
                        pop_filler()
                    if pair < 2:
                        qkt = qkt_n

                for t in range(8, 16):
                    emit_proj(t)

                # ---- LN2 + MLP ----
                h2T = per.tile([128, N_CHUNKS, N_TOK], BF16, tag="ht", bufs=2)
                layernorm_to(h2T, "2")
                mlpT = per.tile([128, N_MCHUNK, 512], BF16, tag="big")
                for n in range(N_TOK // 512):
                    for m in range(N_MCHUNK):
                        pm = psA.tile([128, 512], F32, tag="gen")
                        for c in range(N_CHUNKS):
                            nc.tensor.matmul(
                                pm, lhsT=w1all[:, c, m * 128:(m + 1) * 128],
                                rhs=h2T[:, c, n * 512:(n + 1) * 512],
                                start=(c == 0), stop=(c == N_CHUNKS - 1))
                        if has["b1"]:
                            nc.scalar.activation(
                                out=mlpT[:, m, :], in_=pm, func=AF.Relu,
                                bias=bias_sb["b1"][:, layer, m:m + 1], scale=1.0)
                        else:
                            nc.scalar.activation(out=mlpT[:, m, :], in_=pm,
                                                 func=AF.Relu, scale=1.0)
                    for dt in range(4):
                        t = n * 4 + dt
                        pw = psA.tile([128, 512], F32, tag="gen")
                        for m in range(N_MCHUNK):
                            nc.tensor.matmul(
                                pw[:, :N_EMBED],
                                lhsT=mlpT[:, m, dt * 128:(dt + 1) * 128],
                                rhs=w2all[:, m, :],
                                start=(m == 0), stop=(m == N_MCHUNK - 1))
                        if has["b2"]:
                            tmp = hp.tile([128, N_EMBED], F32, tag="h")
                            nc.vector.tensor_tensor(out=tmp, in0=pw[:, :N_EMBED],
                                                    in1=bias_sb["b2"][:, layer, :],
                                                    op=OP.add)
                            nc.vector.tensor_tensor(out=x[:, t, :], in0=tmp,
                                                    in1=x[:, t, :], op=OP.add)
                        else:
                            nc.vector.tensor_tensor(out=x[:, t, :],
                                                    in0=pw[:, :N_EMBED],
                                                    in1=x[:, t, :], op=OP.add)

            # ---- final LN interleaved with LM head ----
            hT = per.tile([128, N_CHUNKS, N_TOK], BF16, tag="xf")
            wlm_c = []
            for c in range(N_CHUNKS):
                w = wts.tile([128, VOCAB], BF16, tag="wlm", bufs=3)
                nc.sync.dma_start(w, d["wlm"][c * 128:(c + 1) * 128, :])
                wlm_c.append(w)
            layernorm_to(hT, "f")
            for t in range(N_TILES):
                pl = psA.tile([128, 512], F32, tag="gen")
                for c in range(N_CHUNKS):
                    nc.tensor.matmul(pl[:, :VOCAB],
                                     lhsT=hT[:, c, t * 128:(t + 1) * 128],
                                     rhs=wlm_c[c],
                                     start=(c == 0), stop=(c == N_CHUNKS - 1))
                lg = wk.tile([128, VOCAB], F32, tag="lg")
                if has["blm"]:
                    nc.vector.tensor_tensor(out=lg, in0=pl[:, :VOCAB],
                                            in1=bias_sb["blm"], op=OP.add)
                else:
                    copy_out(lg, pl[:, :VOCAB])
                nc.sync.dma_start(logits_d[t * 128:(t + 1) * 128, :], lg)

    nc.compile()
    return nc


_CACHE = {}


def _get_nc(has):
    key = tuple(sorted(has.items()))
    if key not in _CACHE:
        _CACHE[key] = _build(has)
    return _CACHE[key]


def kernel(**inputs):
    shared, has, idx_f = _prep(inputs)
    nc = _get_nc(has)
    in_maps = []
    for core in range(N_CORES):
        m = dict(shared)
        m["idxf"] = idx_f[core]
        in_maps.append(m)
    res = run_bass_kernel_spmd(nc, in_maps, core_ids=list(range(N_CORES)))
    out = np.stack([r["logits"].reshape(B_LOC, T, VOCAB) for r in res.results])
    return out.reshape(B, T, VOCAB)
